# revision 1
# baseline (speedup 1.0000x reference)
"""Trainium2 Bass kernel for nn_CPCModel (CPC-style NCE loss).

Strategy (8 NeuronCores, full inputs on every core, no collectives):

The reference's leave-one-out softmax pooling collapses algebraically:
    pooled[i] = (T - e_i * zt_i) / (S - e_i),  e = exp(s), S = sum(e), T = sum(e_j zt_j)
so the [B,B] pooling matrix is never materialized.  The loss needs only
    nce = -mean_i( total[i,i] - logsumexp_j total[i,j] )
with  total[i, j in group g] = Azw_g[i]·pooled_g[j] + Czw[i]·c[j] + delta_g[i]
where Azw_g = zw @ Ww_g, Czw = zw @ Wk_w, delta_g = zw @ (Ww_g_b + Wk_b).

Each core redundantly computes the cheap pooling prep for all 4096 rows
(no collectives) and computes its own 512 rows of the [4096,4096] total
matrix + row-wise sum(exp(total - 44)); the diagonal comes from an
elementwise product.  Host sums 8x[128,4] partial row values.

Dtypes: the big matmuls (U = [Czw;Azw_g] builds and the 512x4096 total)
run fp32r (full-rate, ~19-bit mantissa).  The small prep matmuls (zt, h,
s, broadcasts, delta, diag partition-sums) run bf16 — the fp32r ISA mode
requires 128 output partitions and even N, which those shapes violate.
Host does layout prep only (transposes / stacking of weights + zw/c).
"""

import numpy as np

import concourse.bacc as bacc
import concourse.bass as bass
import concourse.mybir as mybir
import concourse.tile as tile
from concourse.bass_utils import run_bass_kernel_spmd

N_CORES = 8
B = 4096
OWN = B // N_CORES            # 512 rows of `total` per core
G = 2048                      # group size
F32 = mybir.dt.float32
F32R = mybir.dt.float32r
BF16 = mybir.dt.bfloat16
AF = mybir.ActivationFunctionType
ALU = mybir.AluOpType
SHIFT = 44.0


def _r(ap):
    return ap.bitcast(F32R)


def _build_program(static_diag=False):
    nc = bacc.Bacc(
        "TRN2",
        target_bir_lowering=False,
        debug=False,
        num_devices=N_CORES,
    )

    def din(name, shape, dt):
        return nc.dram_tensor(name, shape, dt, kind="ExternalInput").ap()

    zwTb_d = din("zwTb", [128, B], BF16)     # concat(zw_0,zw_1).T in bf16
    zwoT_d = din("zwoT", [128, OWN], F32R)   # own 512 rows of zw, transposed
    zwoTb_d = din("zwoTb", [128, OWN], BF16)
    cT_d = din("cT", [64, B], F32R)          # c.T
    uw0_d = din("UW0", [128, 128], F32R)     # hstack(Wk_w, Ww0_w)
    uw1_d = din("UW1", [128, 128], F32R)     # hstack(Wk_w, Ww1_w)
    uwo_d = din("UWo", [128, 128], F32R)     # hstack(Wk_w, Ww_{g(core)})
    lwT0_d = din("lwT0", [128, 64], BF16)    # lin0_w.T
    lwT1_d = din("lwT1", [128, 64], BF16)    # lin1_w.T
    a1wB_d = din("a1wB", [128, 64], BF16)    # blockdiag(a0_1w.T, a1_1w.T)
    a2wB_d = din("a2wB", [64, 2], BF16)      # blockdiag(a0_2w.T, a1_2w.T)
    b0_d = din("b0", [128, 1], BF16)         # Ww0_b + Wk_b
    b1_d = din("b1", [128, 1], BF16)         # Ww1_b + Wk_b
    bo_d = din("b_own", [128, 1], BF16)      # b_{group(core)}
    sel2_d = din("sel2", [2, 128], BF16)     # [[1]*64+[0]*64, [0]*64+[1]*64]
    ones_d = din("ones", [128, 1], BF16)
    linb2_d = din("linb2", [128, 1], F32)    # [lin0_b ; lin1_b]
    a1b2_d = din("a1b2", [64, 1], F32)       # [a0_1b ; a1_1b]
    v_d = nc.dram_tensor("v", [128, 4], F32, kind="ExternalOutput").ap()

    from contextlib import ExitStack
    with tile.TileContext(nc) as tc, ExitStack() as ctx:
        pers = ctx.enter_context(tc.tile_pool(name="pers", bufs=1))
        scr = ctx.enter_context(tc.tile_pool(name="scr", bufs=2))
        pbig = ctx.enter_context(tc.tile_pool(name="pbig", bufs=2, space="PSUM"))
        psml = ctx.enter_context(tc.tile_pool(name="psml", bufs=3, space="PSUM"))
        ptin = ctx.enter_context(tc.tile_pool(name="ptin", bufs=1, space="PSUM"))

        def load(name, shape, src, dt):
            t = pers.tile(shape, dt, tag=name, name=name)
            nc.sync.dma_start(t[:], src[:])
            return t

        zwTb = load("zwTb", [128, B], zwTb_d, BF16)
        zwoT = load("zwoT", [128, OWN], zwoT_d, F32R)
        zwoTb = load("zwoTb", [128, OWN], zwoTb_d, BF16)
        uw0_s = load("uw0_s", [128, 128], uw0_d, F32R)
        uw1_s = load("uw1_s", [128, 128], uw1_d, F32R)
        uwo_s = load("uwo_s", [128, 128], uwo_d, F32R)
        lwT0 = load("lwT0", [128, 64], lwT0_d, BF16)
        lwT1 = load("lwT1", [128, 64], lwT1_d, BF16)
        lwT = [lwT0, lwT1]
        a1wB = load("a1wB", [128, 64], a1wB_d, BF16)
        a2wB = load("a2wB", [64, 2], a2wB_d, BF16)
        b0_s = load("b0_s", [128, 1], b0_d, BF16)
        b1_s = load("b1_s", [128, 1], b1_d, BF16)
        bo_s = load("bo_s", [128, 1], bo_d, BF16)
        sel2 = load("sel2", [2, 128], sel2_d, BF16)
        ones = load("ones", [128, 1], ones_d, BF16)
        linb2 = load("linb2", [128, 1], linb2_d, F32)
        a1b2 = load("a1b2", [64, 1], a1b2_d, F32)

        # ---------- V [128, 4096]: rows 0:64 = cT (direct), 64:128 = pooledT ----------
        V = pers.tile([128, B], F32R, tag="V")
        nc.sync.dma_start(V[0:64, :], cT_d[:])

        # ---------- U_g = [Czw ; Azw_g] via one stacked-weight matmul each ----------
        U0 = pers.tile([128, OWN], F32R, tag="U0")
        U1 = pers.tile([128, OWN], F32R, tag="U1")
        UOwn = pers.tile([128, OWN], F32R, tag="UOwn")
        for U, uw in [(U0, uw0_s), (U1, uw1_s), (UOwn, uwo_s)]:
            pu = psml.tile([128, 512], F32, tag="ps")
            nc.tensor.matmul(pu[:], uw[:], zwoT[:], start=True, stop=True)
            nc.vector.tensor_copy(U[:], pu[:])

        # ---------- delta bias columns: biasS[:, g*4+ic] = zw_own[ic]·b_g - SHIFT ----------
        biasS = pers.tile([128, 8], F32, tag="biasS")
        for g, bg in enumerate([b0_s, b1_s]):
            for ic in range(4):
                pd = ptin.tile([128, 1], F32, tag="pt")
                nc.tensor.matmul(pd[:], zwoTb[:, ic * 128:(ic + 1) * 128], bg[:],
                                 start=True, stop=True)
                nc.scalar.activation(biasS[:, g * 4 + ic:g * 4 + ic + 1], pd[:],
                                     AF.Copy, bias=-SHIFT)

        # ---------- ztT2 [128, 2048] bf16: zt0T on 0:64, zt1T on 64:128 ----------
        ztT2 = pers.tile([128, G], BF16, tag="ztT2")
        for ch in range(4):
            pz = psml.tile([128, 512], F32, tag="ps")
            sl = slice(ch * 512, (ch + 1) * 512)
            nc.tensor.matmul(pz[0:64, :], lwT[0][:], zwTb[:, sl],
                             start=True, stop=True)
            nc.tensor.matmul(pz[64:128, :], lwT[1][:],
                             zwTb[:, G + ch * 512:G + (ch + 1) * 512],
                             start=True, stop=True)
            # relu(x + bias) on DVE: (psum add linb2) max 0
            nc.vector.tensor_scalar(ztT2[:, sl], pz[:], linb2[:], 0.0,
                                    op0=ALU.add, op1=ALU.max)

        # ---------- hT2 [64, 2048] bf16: tanh(zt @ a1w.T + b), block-diag ----------
        hT2 = pers.tile([64, G], BF16, tag="hT2")
        for ch in range(4):
            ph = psml.tile([128, 512], F32, tag="ps")
            sl = slice(ch * 512, (ch + 1) * 512)
            nc.tensor.matmul(ph[0:64, :], a1wB[:], ztT2[:, sl],
                             start=True, stop=True)
            nc.scalar.activation(hT2[:, sl], ph[0:64, :], AF.Tanh, bias=a1b2[:])

        # ---------- scores -> eT2 [2, 2048] bf16, S2 [2,1] f32 ----------
        eT2 = pers.tile([2, G], BF16, tag="eT2")
        Sacc = pers.tile([2, 4], F32, tag="Sacc")
        for ch in range(4):
            ps_ = psml.tile([128, 512], F32, tag="ps")
            sl = slice(ch * 512, (ch + 1) * 512)
            nc.tensor.matmul(ps_[0:2, :], a2wB[:], hT2[:, sl],
                             start=True, stop=True)
            nc.scalar.activation(eT2[:, sl], ps_[0:2, :], AF.Exp,
                                 accum_out=Sacc[:, ch:ch + 1])
        S2 = pers.tile([2, 1], F32, tag="S2")
        nc.vector.reduce_sum(S2[:], Sacc[:], axis=mybir.AxisListType.X)

        # ---------- betaT2 = 1/(e - S)  (= -1/(S - e)) ----------
        bT2a = pers.tile([2, G], F32, tag="bT2a")
        nc.vector.tensor_scalar(bT2a[:], eT2[:], S2[:], None, op0=ALU.subtract)
        bT2 = pers.tile([2, G], BF16, tag="bT2")
        with nc.allow_low_precision(reason="beta in bf16 for PE outer-product"):
            nc.vector.reciprocal(bT2[:], bT2a[:])

        # ---------- ztw = zt * e_bcast (ttr also accumulates T), then pooled ----------
        ztwT2 = pers.tile([128, G], F32, tag="ztwT2")
        Tacc = pers.tile([128, 4], F32, tag="Tacc")
        for ch in range(4):
            sl = slice(ch * 512, (ch + 1) * 512)
            peb = psml.tile([128, 512], F32, tag="ps")
            nc.tensor.matmul(peb[:], sel2[:], eT2[:, sl], start=True, stop=True)
            nc.vector.tensor_tensor(ztwT2[:, sl], ztT2[:, sl], peb[:],
                                    op=ALU.mult)
            nc.vector.reduce_sum(Tacc[:, ch:ch + 1], ztwT2[:, sl],
                                 axis=mybir.AxisListType.X)
        T2 = pers.tile([128, 1], F32, tag="T2")
        nc.vector.reduce_sum(T2[:], Tacc[:], axis=mybir.AxisListType.X)

        # pooled = (ztw - T) * beta_bcast   (beta = -1/(S-e) so signs cancel)
        pooled2 = pers.tile([128, G], F32, tag="pooled2")
        for ch in range(4):
            sl = slice(ch * 512, (ch + 1) * 512)
            pbb = psml.tile([128, 512], F32, tag="ps")
            nc.tensor.matmul(pbb[:], sel2[:], bT2[:, sl], start=True, stop=True)
            nc.vector.scalar_tensor_tensor(
                out=pooled2[:, sl], in0=ztwT2[:, sl], scalar=T2[:], in1=pbb[:],
                op0=ALU.subtract, op1=ALU.mult)

        # V rows 64:128: group1 pooled at cols 2048:4096 (converting copy),
        # group0 via partition-shifting sbuf->sbuf DMA (bit-identical f32).
        nc.vector.tensor_copy(V[64:128, G:B], pooled2[64:128, :])
        nc.sync.dma_start(V[64:128, 0:G], _r(pooled2[0:64, :]))

        # ---------- main loop: total rows (own 512) x all 4096 cols ----------
        seacc = pers.tile([128, 16], F32, tag="seacc")
        for ic in range(4):
            usl = slice(ic * 128, (ic + 1) * 128)
            for pair in range(4):
                g = pair // 2
                U = U0 if g == 0 else U1
                pm = pbig.tile([128, 1024], F32, tag="pb")
                for half in range(2):
                    jt = pair * 2 + half
                    nc.tensor.matmul(
                        pm[:, half * 512:(half + 1) * 512],
                        U[:, usl],
                        V[:, jt * 512:(jt + 1) * 512],
                        start=True, stop=True)
                es = scr.tile([128, 1024], BF16, tag="escr")
                nc.scalar.activation(
                    es[:], pm[:], AF.Exp,
                    bias=biasS[:, g * 4 + ic:g * 4 + ic + 1],
                    accum_out=seacc[:, ic * 4 + pair:ic * 4 + pair + 1])

        seall = pers.tile([128, 4], F32, tag="seall")
        for ic in range(4):
            nc.vector.reduce_sum(seall[:, ic:ic + 1], seacc[:, ic * 4:(ic + 1) * 4],
                                 axis=mybir.AxisListType.X)
        lnall = pers.tile([128, 4], F32, tag="lnall")
        nc.scalar.activation(lnall[:], seall[:], AF.Ln)

        # ---------- diagonal: diag[i] = UOwn[:,i]·V[:,own_pos(i)] ----------
        if static_diag:
            vsl = slice(0, OWN)
        else:
            pid = nc.vector.partition_id()
            vsl = bass.ts(pid, OWN)
        prod2 = pers.tile([128, OWN], BF16, tag="prod2")
        nc.vector.tensor_tensor(prod2[:], UOwn[:].bitcast(F32),
                                V[:, vsl].bitcast(F32), op=ALU.mult)

        vall = pers.tile([128, 4], F32, tag="vall")
        for ic in range(4):
            pdg = ptin.tile([128, 1], F32, tag="pt")
            nc.tensor.matmul(pdg[:], prod2[:, ic * 128:(ic + 1) * 128], ones[:],
                             start=True, stop=False)
            nc.tensor.matmul(pdg[:], zwoTb[:, ic * 128:(ic + 1) * 128], bo_s[:],
                             start=False, stop=True)
            # v = (diag_raw + delta - 44) - ln(sumexp)
            nc.vector.scalar_tensor_tensor(
                out=vall[:, ic:ic + 1], in0=pdg[:], scalar=-SHIFT,
                in1=lnall[:, ic:ic + 1], op0=ALU.add, op1=ALU.subtract)

        nc.sync.dma_start(v_d[:], vall[:])

    nc.compile()
    return nc


_built = None


def _get_program():
    global _built
    if _built is None:
        _built = _build_program()
    return _built


def make_in_maps(inputs):
    import ml_dtypes
    BF = ml_dtypes.bfloat16
    f = lambda x: np.ascontiguousarray(np.asarray(x, dtype=np.float32))
    bf = lambda x: np.ascontiguousarray(np.asarray(x, np.float32).astype(BF))

    zw = np.concatenate([f(inputs['zw_0']), f(inputs['zw_1'])], axis=0)
    zwT = np.ascontiguousarray(zw.T)
    b0 = f(inputs['Ww0_b']) + f(inputs['Wk_b'])
    b1 = f(inputs['Ww1_b']) + f(inputs['Wk_b'])

    a1wB = np.zeros((128, 64), np.float32)
    a1wB[0:64, 0:32] = f(inputs['a0_1w']).T
    a1wB[64:128, 32:64] = f(inputs['a1_1w']).T
    a2wB = np.zeros((64, 2), np.float32)
    a2wB[0:32, 0:1] = f(inputs['a0_2w']).T
    a2wB[32:64, 1:2] = f(inputs['a1_2w']).T
    sel2 = np.zeros((2, 128), np.float32)
    sel2[0, 0:64] = 1.0
    sel2[1, 64:128] = 1.0
    linb2 = np.concatenate([f(inputs['lin0_b']), f(inputs['lin1_b'])])
    a1b2 = np.concatenate([f(inputs['a0_1b']), f(inputs['a1_1b'])])
    wk = f(inputs['Wk_w'])
    uw0 = np.hstack([wk, f(inputs['Ww0_w'])])   # [128,128]
    uw1 = np.hstack([wk, f(inputs['Ww1_w'])])

    base = {
        'zwTb': bf(zwT),
        'cT': np.ascontiguousarray(f(inputs['c']).T),
        'UW0': uw0,
        'UW1': uw1,
        'lwT0': bf(f(inputs['lin0_w']).T),
        'lwT1': bf(f(inputs['lin1_w']).T),
        'a1wB': bf(a1wB),
        'a2wB': bf(a2wB),
        'b0': bf(b0.reshape(128, 1)),
        'b1': bf(b1.reshape(128, 1)),
        'sel2': bf(sel2),
        'ones': bf(np.ones((128, 1), np.float32)),
        'linb2': linb2.reshape(128, 1),
        'a1b2': a1b2.reshape(64, 1),
    }
    in_maps = []
    for cid in range(N_CORES):
        g = cid // 4
        m = dict(base)
        zo = np.ascontiguousarray(zwT[:, cid * OWN:(cid + 1) * OWN])
        m['zwoT'] = zo
        m['zwoTb'] = bf(zo)
        m['UWo'] = uw0 if g == 0 else uw1
        m['b_own'] = bf((b0 if g == 0 else b1).reshape(128, 1))
        in_maps.append(m)
    return in_maps


def kernel(**inputs):
    nc = _get_program()
    in_maps = make_in_maps(inputs)
    res = run_bass_kernel_spmd(nc, in_maps, list(range(N_CORES)))
    tot = 0.0
    for r in res.results:
        tot += np.asarray(r['v'], dtype=np.float64).sum()
    return np.array(-(tot / B), dtype=np.float32)



# revision 9
# speedup vs baseline: 1.2974x; 1.2974x over previous
"""Trainium2 Bass kernel for nn_CPCModel (CPC-style NCE loss), v2.

Strategy (8 NeuronCores, full inputs on every core, no collectives):

Leave-one-out softmax pooling collapses algebraically:
    pooled[j] = (T - e_j zt_j)/(S - e_j),  e = exp(s), S = sum e, T = sum e zt
so the [B,B] pooling matrix is never materialized.  The loss needs only
    nce = -mean_i( total[i,i] - logsumexp_j total[i,j] )
with  total[i, j in group g] = Azw_g[i]*pooled_g[j] + Czw[i]*c[j] + delta_g[i].

v2 layout choices (all bf16 matmul path, logits pre-scaled by A_SCALE):
 - Two moving tiles, no partition-shift DMA:
     VA [128,2048] = [pooled0 (parts 0:64) ; cT0 (parts 64:128)]  (group-0 cols)
     VB [128,2048] = [cT1 (parts 0:64) ; pooled1 (parts 64:128)]  (group-1 cols)
   with U0 = A*hstack(Ww0,Wk), U1 = A*hstack(Wk,Ww1) matching each K-order.
 - Scores kept in [8,512] layout (row 2*ch+g = chunk ch, group g) so the
   beta chain (e-S, reciprocal) runs on free-size 512 not 2048.
 - ztw STT carries T via free accum_out; GPSIMD does the diagonal
   partition-sum (axis=C) so the diag path needs no PSUM/PE.
 - Main loop: 8 PSUM tiles [128,2048]; most exp'd on ScalarE LUT
   (scale=1/A), some tiles optionally on DVE via Schraudolph int32 bit-trick.
 - Device returns raw row-sums + diag pieces; host does ln + final sum.
"""

import numpy as np

import concourse.bacc as bacc
import concourse.bass as bass
import concourse.mybir as mybir
import concourse.tile as tile
from concourse.bass_utils import run_bass_kernel_spmd

N_CORES = 8
B = 4096
OWN = B // N_CORES            # 512 rows of `total` per core
G = 2048                      # group size
F32 = mybir.dt.float32
BF16 = mybir.dt.bfloat16
I32 = mybir.dt.int32
AF = mybir.ActivationFunctionType
ALU = mybir.AluOpType
AX = mybir.AxisListType
SHIFT = 44.0

# Schraudolph exp: exp(x) ~= bitcast_f32(int32(A_SCALE*x + B_BIAS)).
A_SCALE = float(np.float32(2.0 ** 23 / np.log(2.0)))   # 12102203.16...
B_BIAS = 1064986316.0   # 127*2^23 - C, C tuned for near-zero mean rel err
INV_A = float(np.float32(1.0 / A_SCALE))

# which of the 8 main tiles use the DVE Schraudolph path (rest: ScalarE LUT)
SCHRAUD_TILES = ()

# blob column layout (bf16 weight blob wbf [128, WBF]):
#   lwT0 0:64 | lwT1 64:128 | a1wB 128:192 | a2wB 192:194 (rows 0:64)
#   sel2 194:322 (rows 0:2) | b01a 322:324 | uw0p 324:452 | uw1 452:580
WBF = 580


def _build_program():
    nc = bacc.Bacc(
        "TRN2",
        target_bir_lowering=False,
        debug=False,
        num_devices=N_CORES,
    )

    def din(name, shape, dt):
        return nc.dram_tensor(name, shape, dt, kind="ExternalInput").ap()

    zwTb_d = din("zwTb", [128, B], BF16)      # chunk-packed zw.T (see host)
    zwoTb_d = din("zwoTb", [128, OWN], BF16)  # own 512 rows of zw, transposed
    cT0_d = din("cT0", [64, G], BF16)         # c.T cols 0:2048
    cT1_d = din("cT1", [64, G], BF16)         # c.T cols 2048:4096
    wbf_d = din("wbf", [128, WBF], BF16)      # packed small weights
    wf32_d = din("wf32", [128, 2], F32)       # linb2 | a1b2
    out1_d = nc.dram_tensor("out1", [128, 16], F32, kind="ExternalOutput").ap()
    dout_d = nc.dram_tensor("dout", [1, 2 * OWN], F32, kind="ExternalOutput").ap()

    from contextlib import ExitStack
    with tile.TileContext(nc) as tc, ExitStack() as ctx:
        pers = ctx.enter_context(tc.tile_pool(name="pers", bufs=1))
        scr = ctx.enter_context(tc.tile_pool(name="scr", bufs=1))

        # ---------------- DMA loads, split across engine rings --------------
        zwTb = pers.tile([128, B], BF16, tag="zwTb", name="zwTb")
        for ch in range(4):
            nc.gpsimd.dma_start(zwTb[:, ch * 1024:(ch + 1) * 1024],
                                zwTb_d[:, ch * 1024:(ch + 1) * 1024])
        wbf = pers.tile([128, WBF], BF16, tag="wbf", name="wbf")
        nc.scalar.dma_start(wbf[:], wbf_d[:])
        wf32 = pers.tile([128, 2], F32, tag="wf32", name="wf32")
        nc.scalar.dma_start(wf32[:], wf32_d[:])
        zwoTb = pers.tile([128, OWN], BF16, tag="zwoTb", name="zwoTb")
        nc.scalar.dma_start(zwoTb[:], zwoTb_d[:])

        VA = pers.tile([128, G], BF16, tag="VA", name="VA")
        VB = pers.tile([128, G], BF16, tag="VB", name="VB")
        nc.sync.dma_start(VA[64:128, :], cT0_d[:])
        nc.sync.dma_start(VB[0:64, :], cT1_d[:])

        lwT0 = wbf[:, 0:64]
        lwT1 = wbf[:, 64:128]
        a1wB = wbf[:, 128:192]
        a2wB = wbf[0:64, 192:194]
        sel2 = wbf[0:2, 194:322]
        b01a = wbf[:, 322:324]
        uw0p = wbf[:, 324:452]
        uw1 = wbf[:, 452:580]
        linb2 = wf32[:, 0:1]
        a1b2 = wf32[0:64, 1:2]

        # persistent SBUF state
        out1 = pers.tile([128, 16], F32, tag="out1", name="out1")
        seacc = out1[:, 0:8]
        biasS = out1[:, 8:16]
        dout = pers.tile([1, 2 * OWN], F32, tag="dout", name="dout")
        biasD = pers.tile([128, 8], F32, tag="biasD", name="biasD")
        ztT2 = pers.tile([128, G], BF16, tag="ztT2", name="ztT2")
        hT2 = pers.tile([64, G], BF16, tag="hT2", name="hT2")
        ztwT2 = pers.tile([128, G], BF16, tag="ztwT2", name="ztwT2")
        e2 = pers.tile([2, G], BF16, tag="e2", name="e2")
        b2 = pers.tile([2, G], BF16, tag="b2", name="b2")
        br2 = pers.tile([2, G], BF16, tag="br2", name="br2")
        Sacc2 = pers.tile([2, 4], F32, tag="Sacc2", name="Sacc2")
        S2 = pers.tile([2, 1], F32, tag="S2", name="S2")
        Tacc = pers.tile([128, 4], F32, tag="Tacc", name="Tacc")
        T2 = pers.tile([128, 1], F32, tag="T2", name="T2")
        U0 = pers.tile([128, OWN], BF16, tag="U0", name="U0")
        U1 = pers.tile([128, OWN], BF16, tag="U1", name="U1")
        bcs = pers.tile([128, G], BF16, tag="bcs", name="bcs")
        prodA = pers.tile([128, OWN], F32, tag="prodA", name="prodA")
        prodB = pers.tile([128, OWN], F32, tag="prodB", name="prodB")

        with tc.tile_pool(name="prep", bufs=1, space="PSUM") as prep:
            def ps(name):
                return prep.tile([128, 512], F32, tag="ps", name=name, bufs=3)

            # ---------------- U builds + delta bias columns ----------------
            for uw, U in ((uw0p, U0), (uw1, U1)):
                pu = ps("pu")
                nc.tensor.matmul(pu[:], uw, zwoTb[:], start=True, stop=True)
                nc.scalar.copy(U[:], pu[:])

            pd = prep.tile([128, 8], F32, tag="mi", name="pd")
            for ic in range(4):
                nc.tensor.matmul(pd[:, 2 * ic:2 * ic + 2],
                                 zwoTb[:, ic * 128:(ic + 1) * 128], b01a,
                                 start=True, stop=True)
            # biasS = delta - 44 (unscaled), biasD = A*delta + (B - 44A)
            nc.scalar.activation(biasS, pd[:], AF.Copy, bias=-SHIFT,
                                 scale=INV_A)
            nc.scalar.activation(biasD[:], pd[:], AF.Copy,
                                 bias=B_BIAS - SHIFT * A_SCALE)

            # ---------------- phase 1: score pipeline + ztw ----------------
            for ch in range(4):
                sl = slice(ch * 512, (ch + 1) * 512)
                pz = ps("pz")
                nc.tensor.matmul(pz[0:64, :], lwT0,
                                 zwTb[:, ch * 1024:ch * 1024 + 512],
                                 start=True, stop=True)
                nc.tensor.matmul(pz[64:128, :], lwT1,
                                 zwTb[:, ch * 1024 + 512:ch * 1024 + 1024],
                                 start=True, stop=True)
                nc.vector.tensor_scalar(ztT2[:, sl], pz[:], linb2, 0.0,
                                        op0=ALU.add, op1=ALU.max)
                ph = ps("ph")
                nc.tensor.matmul(ph[0:64, :], a1wB, ztT2[:, sl],
                                 start=True, stop=True)
                nc.scalar.activation(hT2[:, sl], ph[0:64, :], AF.Tanh,
                                     bias=a1b2)
                s2 = prep.tile([2, 512], F32, tag="s2", name="s2", bufs=2)
                nc.tensor.matmul(s2[:], a2wB, hT2[:, sl],
                                 start=True, stop=True)
                nc.scalar.activation(e2[:, sl], s2[:], AF.Exp,
                                     accum_out=Sacc2[:, ch:ch + 1])
                ebc = prep.tile([128, 512], F32, tag="bc", name="ebc", bufs=2)
                nc.tensor.matmul(ebc[:], sel2, e2[:, sl],
                                 start=True, stop=True)
                nc.vector.scalar_tensor_tensor(
                    out=ztwT2[:, sl], in0=ztT2[:, sl], scalar=0.0, in1=ebc[:],
                    op0=ALU.bypass, op1=ALU.mult,
                    accum_out=Tacc[:, ch:ch + 1])

            # ---------------- phase 2: S, T, beta, pooled -> VA/VB ---------
            nc.vector.reduce_sum(T2[:], Tacc[:], axis=AX.X)
            nc.vector.reduce_sum(S2[:], Sacc2[:], axis=AX.X)
            nc.vector.tensor_scalar(br2[:], e2[:], S2[:], None,
                                    op0=ALU.subtract)
            with nc.allow_low_precision(reason="beta bf16"):
                nc.vector.reciprocal(b2[:], br2[:])
            for ch in range(4):
                sl = slice(ch * 512, (ch + 1) * 512)
                bbc = prep.tile([128, 512], F32, tag="bc", name="bbc", bufs=2)
                nc.tensor.matmul(bbc[:], sel2, b2[:, sl],
                                 start=True, stop=True)
                nc.scalar.copy(bcs[:, sl], bbc[:])
                nc.vector.scalar_tensor_tensor(
                    out=VA[0:64, sl], in0=ztwT2[0:64, sl], scalar=T2[0:64, :],
                    in1=bcs[0:64, sl], op0=ALU.subtract, op1=ALU.mult)
                nc.vector.scalar_tensor_tensor(
                    out=VB[64:128, sl], in0=ztwT2[64:128, sl],
                    scalar=T2[64:128, :],
                    in1=bcs[64:128, sl], op0=ALU.subtract, op1=ALU.mult)

        # ---------------- diagonal pieces (no PSUM needed) -----------------
        pid = nc.vector.partition_id()
        vsl = bass.ts(pid % 4, OWN)
        nc.vector.tensor_tensor(prodA[:], U0[:], VA[:, vsl], op=ALU.mult)
        nc.vector.tensor_tensor(prodB[:], U1[:], VB[:, vsl], op=ALU.mult)
        nc.gpsimd.reduce_sum(dout[0:1, 0:OWN], prodA[:], axis=AX.C)
        nc.gpsimd.reduce_sum(dout[0:1, OWN:2 * OWN], prodB[:], axis=AX.C)

        # ---------------- main loop: 8 tiles of [128 rows x 2048 cols] -----
        with tc.tile_pool(name="mainp", bufs=2, space="PSUM") as mainp:
            for t in range(8):
                ic, g = t // 2, t % 2
                U = U0 if g == 0 else U1
                V = VA if g == 0 else VB
                pm = mainp.tile([128, G], F32, tag="pb", name="pm")
                for q in range(4):
                    nc.tensor.matmul(pm[:, q * 512:(q + 1) * 512],
                                     U[:, ic * 128:(ic + 1) * 128],
                                     V[:, q * 512:(q + 1) * 512],
                                     start=True, stop=True)
                if t in SCHRAUD_TILES:
                    esi = scr.tile([128, G], I32, tag="esi", name="esi")
                    nc.vector.tensor_scalar(esi[:], pm[:],
                                            biasD[:, t:t + 1], None,
                                            op0=ALU.add)
                    nc.vector.reduce_sum(seacc[:, t:t + 1],
                                         esi[:].bitcast(F32), axis=AX.X)
                else:
                    es = scr.tile([128, G], BF16, tag="es", name="es")
                    nc.scalar.activation(es[:], pm[:], AF.Exp,
                                         bias=biasS[:, t:t + 1], scale=INV_A,
                                         accum_out=seacc[:, t:t + 1])

        nc.sync.dma_start(out1_d[:], out1[:])
        nc.sync.dma_start(dout_d[:], dout[:])

    nc.compile()
    return nc


_built = None


def _get_program():
    global _built
    if _built is None:
        _built = _build_program()
    return _built


def make_in_maps(inputs):
    import ml_dtypes
    BF = ml_dtypes.bfloat16
    f = lambda x: np.ascontiguousarray(np.asarray(x, dtype=np.float32))
    bf = lambda x: np.ascontiguousarray(np.asarray(x, np.float32).astype(BF))

    zw = np.concatenate([f(inputs['zw_0']), f(inputs['zw_1'])], axis=0)
    zwT = np.ascontiguousarray(zw.T)                      # [128, 4096]
    # chunk-packed: block ch = [g0 cols ch*512:+512 | g1 cols ch*512:+512]
    zwTbCH = np.empty((128, B), np.float32)
    for ch in range(4):
        zwTbCH[:, ch * 1024:ch * 1024 + 512] = zwT[:, ch * 512:(ch + 1) * 512]
        zwTbCH[:, ch * 1024 + 512:ch * 1024 + 1024] = \
            zwT[:, G + ch * 512:G + (ch + 1) * 512]

    wk = f(inputs['Wk_w'])
    b0 = f(inputs['Ww0_b']) + f(inputs['Wk_b'])
    b1 = f(inputs['Ww1_b']) + f(inputs['Wk_b'])

    a1wB = np.zeros((128, 64), np.float32)
    a1wB[0:64, 0:32] = f(inputs['a0_1w']).T
    a1wB[64:128, 32:64] = f(inputs['a1_1w']).T
    a2wB = np.zeros((64, 2), np.float32)
    a2wB[0:32, 0:1] = f(inputs['a0_2w']).T
    a2wB[32:64, 1:2] = f(inputs['a1_2w']).T
    sel2 = np.zeros((2, 128), np.float32)
    sel2[0, 0:64] = 1.0
    sel2[1, 64:128] = 1.0

    wbf = np.zeros((128, WBF), np.float32)
    wbf[:, 0:64] = f(inputs['lin0_w']).T
    wbf[:, 64:128] = f(inputs['lin1_w']).T
    wbf[:, 128:192] = a1wB
    wbf[0:64, 192:194] = a2wB
    wbf[0:2, 194:322] = sel2
    wbf[:, 322:323] = (A_SCALE * b0).reshape(128, 1)
    wbf[:, 323:324] = (A_SCALE * b1).reshape(128, 1)
    wbf[:, 324:452] = A_SCALE * np.hstack([f(inputs['Ww0_w']), wk])
    wbf[:, 452:580] = A_SCALE * np.hstack([wk, f(inputs['Ww1_w'])])

    wf32 = np.zeros((128, 2), np.float32)
    wf32[:, 0] = np.concatenate([f(inputs['lin0_b']), f(inputs['lin1_b'])])
    wf32[0:64, 1] = np.concatenate([f(inputs['a0_1b']), f(inputs['a1_1b'])])

    cT = f(inputs['c']).T                                  # [64, 4096]

    base = {
        'zwTb': bf(zwTbCH),
        'cT0': bf(cT[:, 0:G]),
        'cT1': bf(cT[:, G:B]),
        'wbf': bf(wbf),
        'wf32': wf32,
    }
    in_maps = []
    for cid in range(N_CORES):
        m = dict(base)
        m['zwoTb'] = bf(zwT[:, cid * OWN:(cid + 1) * OWN])
        in_maps.append(m)
    return in_maps


def kernel(**inputs):
    nc = _get_program()
    in_maps = make_in_maps(inputs)
    res = run_bass_kernel_spmd(nc, in_maps, list(range(N_CORES)))
    total = 0.0
    for cid, r in enumerate(res.results):
        g = cid // 4
        out1 = np.asarray(r['out1'], dtype=np.float64)   # [128,16]
        dout = np.asarray(r['dout'], dtype=np.float64).reshape(2, 512)
        seacc, biasS = out1[:, 0:8], out1[:, 8:16]
        diag_raw = dout[g, :] / A_SCALE                  # [512] own rows
        for ic in range(4):
            p = np.arange(128)
            u = ic * 128 + p
            sumexp = seacc[p, 2 * ic] + seacc[p, 2 * ic + 1]
            v = diag_raw[u] + biasS[p, 2 * ic + g] - np.log(sumexp)
            total += v.sum()
    return np.array(-(total / B), dtype=np.float32)


# revision 17
# speedup vs baseline: 1.4795x; 1.1403x over previous
"""Trainium2 Bass kernel for nn_CPCModel (CPC-style NCE loss), v2.

Strategy (8 NeuronCores, full inputs on every core, no collectives):

Leave-one-out softmax pooling collapses algebraically:
    pooled[j] = (T - e_j zt_j)/(S - e_j),  e = exp(s), S = sum e, T = sum e zt
so the [B,B] pooling matrix is never materialized.  The loss needs only
    nce = -mean_i( total[i,i] - logsumexp_j total[i,j] )
with  total[i, j in group g] = Azw_g[i]*pooled_g[j] + Czw[i]*c[j] + delta_g[i].

v2 layout choices (all bf16 matmul path, logits pre-scaled by A_SCALE):
 - Two moving tiles, no partition-shift DMA:
     VA [128,2048] = [pooled0 (parts 0:64) ; cT0 (parts 64:128)]  (group-0 cols)
     VB [128,2048] = [cT1 (parts 0:64) ; pooled1 (parts 64:128)]  (group-1 cols)
   with U0 = A*hstack(Ww0,Wk), U1 = A*hstack(Wk,Ww1) matching each K-order.
 - Scores kept in [8,512] layout (row 2*ch+g = chunk ch, group g) so the
   beta chain (e-S, reciprocal) runs on free-size 512 not 2048.
 - ztw STT carries T via free accum_out; GPSIMD does the diagonal
   partition-sum (axis=C) so the diag path needs no PSUM/PE.
 - Main loop: 8 PSUM tiles [128,2048]; most exp'd on ScalarE LUT
   (scale=1/A), some tiles optionally on DVE via Schraudolph int32 bit-trick.
 - Device returns raw row-sums + diag pieces; host does ln + final sum.
"""

import numpy as np

import concourse.bacc as bacc
import concourse.bass as bass
import concourse.mybir as mybir
import concourse.tile as tile
from concourse.bass_utils import run_bass_kernel_spmd

N_CORES = 8
B = 4096
OWN = B // N_CORES            # 512 rows of `total` per core
G = 2048                      # group size
F32 = mybir.dt.float32
BF16 = mybir.dt.bfloat16
I32 = mybir.dt.int32
AF = mybir.ActivationFunctionType
ALU = mybir.AluOpType
AX = mybir.AxisListType
SHIFT = 44.0

# Schraudolph exp: exp(x) ~= bitcast_f32(int32(A_SCALE*x + B_BIAS)).
A_SCALE = float(np.float32(2.0 ** 23 / np.log(2.0)))   # 12102203.16...
B_BIAS = 1064986316.0   # 127*2^23 - C, C tuned for near-zero mean rel err
INV_A = float(np.float32(1.0 / A_SCALE))

# which of the 8 main tiles use the DVE Schraudolph path (rest: ScalarE LUT)
SCHRAUD_TILES = ()

# blob column layout (bf16 weight blob wbf [128, WBF]):
#   lwT0 0:64 | lwT1 64:128 | a1wB 128:192 | a2wB 192:194 (rows 0:64)
#   sel2 194:322 (rows 0:2) | b01a 322:324 | uw0p 324:452 | uw1 452:580
WBF = 580


def _build_program():
    nc = bacc.Bacc(
        "TRN2",
        target_bir_lowering=False,
        debug=False,
        num_devices=N_CORES,
    )

    def din(name, shape, dt):
        return nc.dram_tensor(name, shape, dt, kind="ExternalInput").ap()

    zwTb_d = din("zwTb", [128, B], BF16)      # chunk-packed zw.T (see host)
    zwoTb_d = din("zwoTb", [128, OWN], BF16)  # own 512 rows of zw, transposed
    cT0_d = din("cT0", [64, G], BF16)         # c.T cols 0:2048
    cT1_d = din("cT1", [64, G], BF16)         # c.T cols 2048:4096
    wbf_d = din("wbf", [128, WBF], BF16)      # packed small weights
    wf32_d = din("wf32", [128, 2], F32)       # linb2 | a1b2
    out1_d = nc.dram_tensor("out1", [128, 16], F32, kind="ExternalOutput").ap()
    dout_d = nc.dram_tensor("dout", [1, 2 * OWN], F32, kind="ExternalOutput").ap()

    from contextlib import ExitStack
    with tile.TileContext(nc) as tc, ExitStack() as ctx:
        pers = ctx.enter_context(tc.tile_pool(name="pers", bufs=1))
        scr = ctx.enter_context(tc.tile_pool(name="scr", bufs=1))

        # ---------------- DMA loads, split across engine rings --------------
        # small blobs first on the idle SP ring (they gate the U builds);
        # zwTb chunks stream on the Pool ring; cT last (needed only by main).
        wbf = pers.tile([128, WBF], BF16, tag="wbf", name="wbf")
        nc.sync.dma_start(wbf[:], wbf_d[:])
        zwoTb = pers.tile([128, OWN], BF16, tag="zwoTb", name="zwoTb")
        nc.sync.dma_start(zwoTb[:], zwoTb_d[:])
        wf32 = pers.tile([128, 2], F32, tag="wf32", name="wf32")
        nc.sync.dma_start(wf32[:], wf32_d[:])
        zwTb = pers.tile([128, B], BF16, tag="zwTb", name="zwTb")
        for ch in range(4):
            nc.gpsimd.dma_start(zwTb[:, ch * 1024:(ch + 1) * 1024],
                                zwTb_d[:, ch * 1024:(ch + 1) * 1024])

        VA = pers.tile([128, G], BF16, tag="VA", name="VA")
        VB = pers.tile([128, G], BF16, tag="VB", name="VB")
        nc.sync.dma_start(VA[64:128, :], cT0_d[:])
        nc.sync.dma_start(VB[0:64, :], cT1_d[:])

        lwT0 = wbf[:, 0:64]
        lwT1 = wbf[:, 64:128]
        a1wB = wbf[:, 128:192]
        a2wB = wbf[0:64, 192:194]
        sel2 = wbf[0:2, 194:322]
        b01a = wbf[:, 322:324]
        uw0p = wbf[:, 324:452]
        uw1 = wbf[:, 452:580]
        linb2 = wf32[:, 0:1]
        a1b2 = wf32[0:64, 1:2]

        # persistent SBUF state
        out1 = pers.tile([128, 16], F32, tag="out1", name="out1")
        seacc = out1[:, 0:8]
        biasS = out1[:, 8:16]
        dout = pers.tile([1, 2 * OWN], F32, tag="dout", name="dout")
        biasD = pers.tile([128, 8], F32, tag="biasD", name="biasD")
        ztT2 = pers.tile([128, G], BF16, tag="ztT2", name="ztT2")
        hT2 = pers.tile([64, G], BF16, tag="hT2", name="hT2")
        ztwT2 = pers.tile([128, G], BF16, tag="ztwT2", name="ztwT2")
        e2 = pers.tile([2, G], BF16, tag="e2", name="e2")
        Sacc2 = pers.tile([2, 4], F32, tag="Sacc2", name="Sacc2")
        S2 = pers.tile([2, 1], F32, tag="S2", name="S2")
        S2b = pers.tile([2, 1], BF16, tag="S2b", name="S2b")
        Sb = pers.tile([128, 1], F32, tag="Sb", name="Sb")
        Tacc = pers.tile([128, 4], F32, tag="Tacc", name="Tacc")
        T2 = pers.tile([128, 1], F32, tag="T2", name="T2")
        U0 = pers.tile([128, OWN], BF16, tag="U0", name="U0")
        U1 = pers.tile([128, OWN], BF16, tag="U1", name="U1")
        bcs = pers.tile([128, G], BF16, tag="bcs", name="bcs")
        bcs1 = pers.tile([128, G], BF16, tag="bcs1", name="bcs1")
        numer = pers.tile([128, G], BF16, tag="numer", name="numer")
        prodA = pers.tile([128, OWN], F32, tag="prodA", name="prodA")
        prodB = pers.tile([128, OWN], F32, tag="prodB", name="prodB")

        with tc.tile_pool(name="prep", bufs=1, space="PSUM") as prep:
            def ps(name):
                return prep.tile([128, 512], F32, tag="ps", name=name, bufs=3)

            # ---------------- U builds + delta bias columns ----------------
            for uw, U in ((uw0p, U0), (uw1, U1)):
                pu = ps("pu")
                nc.tensor.matmul(pu[:], uw, zwoTb[:], start=True, stop=True)
                nc.scalar.copy(U[:], pu[:])

            pd = prep.tile([128, 8], F32, tag="mi", name="pd")
            for ic in range(4):
                nc.tensor.matmul(pd[:, 2 * ic:2 * ic + 2],
                                 zwoTb[:, ic * 128:(ic + 1) * 128], b01a,
                                 start=True, stop=True)
            # biasS = delta - 44 (unscaled), biasD = A*delta + (B - 44A)
            nc.scalar.activation(biasS, pd[:], AF.Copy, bias=-SHIFT,
                                 scale=INV_A)
            nc.scalar.activation(biasD[:], pd[:], AF.Copy,
                                 bias=B_BIAS - SHIFT * A_SCALE)

            # ---------------- phase 1: score pipeline + ztw ----------------
            for ch in range(4):
                sl = slice(ch * 512, (ch + 1) * 512)
                pz = ps("pz")
                nc.tensor.matmul(pz[0:64, :], lwT0,
                                 zwTb[:, ch * 1024:ch * 1024 + 512],
                                 start=True, stop=True)
                nc.tensor.matmul(pz[64:128, :], lwT1,
                                 zwTb[:, ch * 1024 + 512:ch * 1024 + 1024],
                                 start=True, stop=True)
                nc.vector.tensor_scalar(ztT2[:, sl], pz[:], linb2, 0.0,
                                        op0=ALU.add, op1=ALU.max)
                ph = ps("ph")
                nc.tensor.matmul(ph[0:64, :], a1wB, ztT2[:, sl],
                                 start=True, stop=True)
                nc.scalar.activation(hT2[:, sl], ph[0:64, :], AF.Tanh,
                                     bias=a1b2)
                s2 = prep.tile([2, 512], F32, tag="s2", name="s2", bufs=2)
                nc.tensor.matmul(s2[:], a2wB, hT2[:, sl],
                                 start=True, stop=True)
                nc.scalar.activation(e2[:, sl], s2[:], AF.Exp,
                                     accum_out=Sacc2[:, ch:ch + 1])
                ebc = prep.tile([128, 512], F32, tag="bc", name="ebc", bufs=2)
                nc.tensor.matmul(ebc[:], sel2, e2[:, sl],
                                 start=True, stop=True)
                # ztwT2 holds MINUS zt*e so the pooled numerator (T - ztw)
                # comes out of one subtract; Tacc accumulates -T.
                nc.vector.scalar_tensor_tensor(
                    out=ztwT2[:, sl], in0=ztT2[:, sl], scalar=-1.0,
                    in1=ebc[:], op0=ALU.mult, op1=ALU.mult,
                    accum_out=Tacc[:, ch:ch + 1])

            # ---------------- phase 2: pooled = (T - ztw)/(S - e) ----------
            nc.vector.reduce_sum(T2[:], Tacc[:], axis=AX.X)   # = -T
            nc.vector.reduce_sum(S2[:], Sacc2[:], axis=AX.X)
            nc.vector.tensor_copy(S2b[:], S2[:])
            Sp = prep.tile([128, 1], F32, tag="mi", name="Sp")
            nc.tensor.matmul(Sp[:], sel2, S2b[:], start=True, stop=True)
            nc.vector.tensor_copy(Sb[:], Sp[:])
            for ch in range(4):
                sl = slice(ch * 512, (ch + 1) * 512)
                bbc = prep.tile([128, 512], F32, tag="bc", name="bbc", bufs=2)
                nc.tensor.matmul(bbc[:], sel2, e2[:, sl],
                                 start=True, stop=True)
                # bcs1 = Relu(S - e) == S - e  (strictly positive)
                nc.scalar.activation(bcs1[:, sl], bbc[:], AF.Relu,
                                     scale=-1.0, bias=Sb[:])
                with nc.allow_low_precision(reason="beta bf16"):
                    nc.vector.reciprocal(bcs[:, sl], bcs1[:, sl])
                # numer = -ztw - (-T) = T - ztw;  pooled = numer/(S - e)
                nc.vector.tensor_scalar(numer[:, sl], ztwT2[:, sl],
                                        T2[:], None, op0=ALU.subtract)
                nc.vector.tensor_tensor(VA[0:64, sl], numer[0:64, sl],
                                        bcs[0:64, sl], op=ALU.mult)
                nc.vector.tensor_tensor(VB[64:128, sl], numer[64:128, sl],
                                        bcs[64:128, sl], op=ALU.mult)

        # ---------------- diagonal pieces (no PSUM needed) -----------------
        pid = nc.vector.partition_id()
        vsl = bass.ts(pid % 4, OWN)
        nc.vector.tensor_tensor(prodA[:], U0[:], VA[:, vsl], op=ALU.mult)
        nc.vector.tensor_tensor(prodB[:], U1[:], VB[:, vsl], op=ALU.mult)
        nc.gpsimd.reduce_sum(dout[0:1, 0:OWN], prodA[:], axis=AX.C)
        nc.gpsimd.reduce_sum(dout[0:1, OWN:2 * OWN], prodB[:], axis=AX.C)

        # ---------------- main loop: 8 tiles of [128 rows x 2048 cols] -----
        with tc.tile_pool(name="mainp", bufs=2, space="PSUM") as mainp:
            for t in range(8):
                ic, g = t // 2, t % 2
                U = U0 if g == 0 else U1
                V = VA if g == 0 else VB
                pm = mainp.tile([128, G], F32, tag="pb", name="pm")
                for q in range(4):
                    nc.tensor.matmul(pm[:, q * 512:(q + 1) * 512],
                                     U[:, ic * 128:(ic + 1) * 128],
                                     V[:, q * 512:(q + 1) * 512],
                                     start=True, stop=True)
                if t in SCHRAUD_TILES:
                    esi = scr.tile([128, G], I32, tag="esi", name="esi")
                    nc.vector.tensor_scalar(esi[:], pm[:],
                                            biasD[:, t:t + 1], None,
                                            op0=ALU.add)
                    nc.vector.reduce_sum(seacc[:, t:t + 1],
                                         esi[:].bitcast(F32), axis=AX.X)
                else:
                    es = scr.tile([128, G], BF16, tag="es", name="es")
                    nc.scalar.activation(es[:], pm[:], AF.Exp,
                                         bias=biasS[:, t:t + 1], scale=INV_A,
                                         accum_out=seacc[:, t:t + 1])

        nc.sync.dma_start(out1_d[:], out1[:])
        nc.sync.dma_start(dout_d[:], dout[:])

    nc.compile()
    return nc


_built = None


def _get_program():
    global _built
    if _built is None:
        _built = _build_program()
    return _built


def make_in_maps(inputs):
    import ml_dtypes
    BF = ml_dtypes.bfloat16
    f = lambda x: np.ascontiguousarray(np.asarray(x, dtype=np.float32))
    bf = lambda x: np.ascontiguousarray(np.asarray(x, np.float32).astype(BF))

    zw = np.concatenate([f(inputs['zw_0']), f(inputs['zw_1'])], axis=0)
    zwT = np.ascontiguousarray(zw.T)                      # [128, 4096]
    # chunk-packed: block ch = [g0 cols ch*512:+512 | g1 cols ch*512:+512]
    zwTbCH = np.empty((128, B), np.float32)
    for ch in range(4):
        zwTbCH[:, ch * 1024:ch * 1024 + 512] = zwT[:, ch * 512:(ch + 1) * 512]
        zwTbCH[:, ch * 1024 + 512:ch * 1024 + 1024] = \
            zwT[:, G + ch * 512:G + (ch + 1) * 512]

    wk = f(inputs['Wk_w'])
    b0 = f(inputs['Ww0_b']) + f(inputs['Wk_b'])
    b1 = f(inputs['Ww1_b']) + f(inputs['Wk_b'])

    a1wB = np.zeros((128, 64), np.float32)
    a1wB[0:64, 0:32] = f(inputs['a0_1w']).T
    a1wB[64:128, 32:64] = f(inputs['a1_1w']).T
    a2wB = np.zeros((64, 2), np.float32)
    a2wB[0:32, 0:1] = f(inputs['a0_2w']).T
    a2wB[32:64, 1:2] = f(inputs['a1_2w']).T
    sel2 = np.zeros((2, 128), np.float32)
    sel2[0, 0:64] = 1.0
    sel2[1, 64:128] = 1.0

    wbf = np.zeros((128, WBF), np.float32)
    wbf[:, 0:64] = f(inputs['lin0_w']).T
    wbf[:, 64:128] = f(inputs['lin1_w']).T
    wbf[:, 128:192] = a1wB
    wbf[0:64, 192:194] = a2wB
    wbf[0:2, 194:322] = sel2
    wbf[:, 322:323] = (A_SCALE * b0).reshape(128, 1)
    wbf[:, 323:324] = (A_SCALE * b1).reshape(128, 1)
    wbf[:, 324:452] = A_SCALE * np.hstack([f(inputs['Ww0_w']), wk])
    wbf[:, 452:580] = A_SCALE * np.hstack([wk, f(inputs['Ww1_w'])])

    wf32 = np.zeros((128, 2), np.float32)
    wf32[:, 0] = np.concatenate([f(inputs['lin0_b']), f(inputs['lin1_b'])])
    wf32[0:64, 1] = np.concatenate([f(inputs['a0_1b']), f(inputs['a1_1b'])])

    cT = f(inputs['c']).T                                  # [64, 4096]

    base = {
        'zwTb': bf(zwTbCH),
        'cT0': bf(cT[:, 0:G]),
        'cT1': bf(cT[:, G:B]),
        'wbf': bf(wbf),
        'wf32': wf32,
    }
    in_maps = []
    for cid in range(N_CORES):
        m = dict(base)
        m['zwoTb'] = bf(zwT[:, cid * OWN:(cid + 1) * OWN])
        in_maps.append(m)
    return in_maps


def kernel(**inputs):
    nc = _get_program()
    in_maps = make_in_maps(inputs)
    res = run_bass_kernel_spmd(nc, in_maps, list(range(N_CORES)))
    total = 0.0
    for cid, r in enumerate(res.results):
        g = cid // 4
        out1 = np.asarray(r['out1'], dtype=np.float64)   # [128,16]
        dout = np.asarray(r['dout'], dtype=np.float64).reshape(2, 512)
        seacc, biasS = out1[:, 0:8], out1[:, 8:16]
        diag_raw = dout[g, :] / A_SCALE                  # [512] own rows
        for ic in range(4):
            p = np.arange(128)
            u = ic * 128 + p
            sumexp = seacc[p, 2 * ic] + seacc[p, 2 * ic + 1]
            v = diag_raw[u] + biasS[p, 2 * ic + g] - np.log(sumexp)
            total += v.sum()
    return np.array(-(total / B), dtype=np.float32)


# revision 19
# speedup vs baseline: 1.5642x; 1.0573x over previous
"""Trainium2 Bass kernel for nn_CPCModel (CPC-style NCE loss), v2.

Strategy (8 NeuronCores, full inputs on every core, no collectives):

Leave-one-out softmax pooling collapses algebraically:
    pooled[j] = (T - e_j zt_j)/(S - e_j),  e = exp(s), S = sum e, T = sum e zt
so the [B,B] pooling matrix is never materialized.  The loss needs only
    nce = -mean_i( total[i,i] - logsumexp_j total[i,j] )
with  total[i, j in group g] = Azw_g[i]*pooled_g[j] + Czw[i]*c[j] + delta_g[i].

v2 layout choices (all bf16 matmul path, logits pre-scaled by A_SCALE):
 - Two moving tiles, no partition-shift DMA:
     VA [128,2048] = [pooled0 (parts 0:64) ; cT0 (parts 64:128)]  (group-0 cols)
     VB [128,2048] = [cT1 (parts 0:64) ; pooled1 (parts 64:128)]  (group-1 cols)
   with U0 = A*hstack(Ww0,Wk), U1 = A*hstack(Wk,Ww1) matching each K-order.
 - Scores kept in [8,512] layout (row 2*ch+g = chunk ch, group g) so the
   beta chain (e-S, reciprocal) runs on free-size 512 not 2048.
 - ztw STT carries T via free accum_out; GPSIMD does the diagonal
   partition-sum (axis=C) so the diag path needs no PSUM/PE.
 - Main loop: 8 PSUM tiles [128,2048]; most exp'd on ScalarE LUT
   (scale=1/A), some tiles optionally on DVE via Schraudolph int32 bit-trick.
 - Device returns raw row-sums + diag pieces; host does ln + final sum.
"""

import numpy as np

import concourse.bacc as bacc
import concourse.bass as bass
import concourse.mybir as mybir
import concourse.tile as tile
from concourse.bass_utils import run_bass_kernel_spmd

N_CORES = 8
B = 4096
OWN = B // N_CORES            # 512 rows of `total` per core
G = 2048                      # group size
F32 = mybir.dt.float32
BF16 = mybir.dt.bfloat16
I32 = mybir.dt.int32
AF = mybir.ActivationFunctionType
ALU = mybir.AluOpType
AX = mybir.AxisListType
SHIFT = 44.0

# Schraudolph exp: exp(x) ~= bitcast_f32(int32(A_SCALE*x + B_BIAS)).
A_SCALE = float(np.float32(2.0 ** 23 / np.log(2.0)))   # 12102203.16...
B_BIAS = 1064869216.0   # 127*2^23 - 484000, tuned on real logit distribution
INV_A = float(np.float32(1.0 / A_SCALE))

# which of the 8 main tiles use the DVE Schraudolph path (rest: ScalarE LUT)
SCHRAUD_TILES = (2, 5)

# blob column layout (bf16 weight blob wbf [128, WBF]):
#   lwT0 0:64 | lwT1 64:128 | a1wB 128:192 | a2wB 192:194 (rows 0:64)
#   sel2 194:322 (rows 0:2) | b01a 322:324 | uw0p 324:452 | uw1 452:580
WBF = 580


def _build_program():
    nc = bacc.Bacc(
        "TRN2",
        target_bir_lowering=False,
        debug=False,
        num_devices=N_CORES,
    )

    def din(name, shape, dt):
        return nc.dram_tensor(name, shape, dt, kind="ExternalInput").ap()

    zwTb_d = din("zwTb", [128, B], BF16)      # chunk-packed zw.T (see host)
    zwoTb_d = din("zwoTb", [128, OWN], BF16)  # own 512 rows of zw, transposed
    cT0_d = din("cT0", [64, G], BF16)         # c.T cols 0:2048
    cT1_d = din("cT1", [64, G], BF16)         # c.T cols 2048:4096
    wbf_d = din("wbf", [128, WBF], BF16)      # packed small weights
    wf32_d = din("wf32", [128, 2], F32)       # linb2 | a1b2
    out1_d = nc.dram_tensor("out1", [128, 16], F32, kind="ExternalOutput").ap()
    dout_d = nc.dram_tensor("dout", [1, 2 * OWN], F32, kind="ExternalOutput").ap()

    from contextlib import ExitStack
    with tile.TileContext(nc) as tc, ExitStack() as ctx:
        pers = ctx.enter_context(tc.tile_pool(name="pers", bufs=1))
        scr = ctx.enter_context(tc.tile_pool(name="scr", bufs=1))

        # ---------------- DMA loads, split across engine rings --------------
        # small blobs first on the idle SP ring (they gate the U builds);
        # zwTb chunks stream on the Pool ring; cT last (needed only by main).
        wbf = pers.tile([128, WBF], BF16, tag="wbf", name="wbf")
        nc.sync.dma_start(wbf[:], wbf_d[:])
        zwoTb = pers.tile([128, OWN], BF16, tag="zwoTb", name="zwoTb")
        nc.sync.dma_start(zwoTb[:], zwoTb_d[:])
        wf32 = pers.tile([128, 2], F32, tag="wf32", name="wf32")
        nc.sync.dma_start(wf32[:], wf32_d[:])
        zwTb = pers.tile([128, B], BF16, tag="zwTb", name="zwTb")
        for ch in range(4):
            nc.gpsimd.dma_start(zwTb[:, ch * 1024:(ch + 1) * 1024],
                                zwTb_d[:, ch * 1024:(ch + 1) * 1024])

        VA = pers.tile([128, G], BF16, tag="VA", name="VA")
        VB = pers.tile([128, G], BF16, tag="VB", name="VB")
        nc.sync.dma_start(VA[64:128, :], cT0_d[:])
        nc.sync.dma_start(VB[0:64, :], cT1_d[:])

        lwT0 = wbf[:, 0:64]
        lwT1 = wbf[:, 64:128]
        a1wB = wbf[:, 128:192]
        a2wB = wbf[0:64, 192:194]
        sel2 = wbf[0:2, 194:322]
        b01a = wbf[:, 322:324]
        uw0p = wbf[:, 324:452]
        uw1 = wbf[:, 452:580]
        linb2 = wf32[:, 0:1]
        a1b2 = wf32[0:64, 1:2]

        # persistent SBUF state
        out1 = pers.tile([128, 16], F32, tag="out1", name="out1")
        seacc = out1[:, 0:8]
        biasS = out1[:, 8:16]
        dout = pers.tile([1, 2 * OWN], F32, tag="dout", name="dout")
        biasD = pers.tile([128, 8], F32, tag="biasD", name="biasD")
        ztT2 = pers.tile([128, G], BF16, tag="ztT2", name="ztT2")
        hT2 = pers.tile([64, G], BF16, tag="hT2", name="hT2")
        ztwT2 = pers.tile([128, G], BF16, tag="ztwT2", name="ztwT2")
        e2 = pers.tile([2, G], BF16, tag="e2", name="e2")
        Sacc2 = pers.tile([2, 4], F32, tag="Sacc2", name="Sacc2")
        S2 = pers.tile([2, 1], F32, tag="S2", name="S2")
        S2b = pers.tile([2, 1], BF16, tag="S2b", name="S2b")
        Sb = pers.tile([128, 1], F32, tag="Sb", name="Sb")
        Tacc = pers.tile([128, 4], F32, tag="Tacc", name="Tacc")
        T2 = pers.tile([128, 1], F32, tag="T2", name="T2")
        U0 = pers.tile([128, OWN], BF16, tag="U0", name="U0")
        U1 = pers.tile([128, OWN], BF16, tag="U1", name="U1")
        bcs = pers.tile([128, G], BF16, tag="bcs", name="bcs")
        bcs1 = pers.tile([128, G], BF16, tag="bcs1", name="bcs1")
        numer = pers.tile([128, G], BF16, tag="numer", name="numer")
        prodA = pers.tile([128, OWN], F32, tag="prodA", name="prodA")
        prodB = pers.tile([128, OWN], F32, tag="prodB", name="prodB")

        with tc.tile_pool(name="prep", bufs=1, space="PSUM") as prep:
            def ps(name):
                return prep.tile([128, 512], F32, tag="ps", name=name, bufs=3)

            # ---------------- U builds + delta bias columns ----------------
            for uw, U in ((uw0p, U0), (uw1, U1)):
                pu = ps("pu")
                nc.tensor.matmul(pu[:], uw, zwoTb[:], start=True, stop=True)
                nc.scalar.copy(U[:], pu[:])

            pd = prep.tile([128, 8], F32, tag="mi", name="pd")
            for ic in range(4):
                nc.tensor.matmul(pd[:, 2 * ic:2 * ic + 2],
                                 zwoTb[:, ic * 128:(ic + 1) * 128], b01a,
                                 start=True, stop=True)
            # biasS = delta - 44 (unscaled), biasD = A*delta + (B - 44A)
            nc.scalar.activation(biasS, pd[:], AF.Copy, bias=-SHIFT,
                                 scale=INV_A)
            nc.scalar.activation(biasD[:], pd[:], AF.Copy,
                                 bias=B_BIAS - SHIFT * A_SCALE)

            # ---------------- phase 1: score pipeline + ztw ----------------
            for ch in range(4):
                sl = slice(ch * 512, (ch + 1) * 512)
                pz = ps("pz")
                nc.tensor.matmul(pz[0:64, :], lwT0,
                                 zwTb[:, ch * 1024:ch * 1024 + 512],
                                 start=True, stop=True)
                nc.tensor.matmul(pz[64:128, :], lwT1,
                                 zwTb[:, ch * 1024 + 512:ch * 1024 + 1024],
                                 start=True, stop=True)
                nc.vector.tensor_scalar(ztT2[:, sl], pz[:], linb2, 0.0,
                                        op0=ALU.add, op1=ALU.max)
                ph = ps("ph")
                nc.tensor.matmul(ph[0:64, :], a1wB, ztT2[:, sl],
                                 start=True, stop=True)
                nc.scalar.activation(hT2[:, sl], ph[0:64, :], AF.Tanh,
                                     bias=a1b2)
                s2 = prep.tile([2, 512], F32, tag="s2", name="s2", bufs=2)
                nc.tensor.matmul(s2[:], a2wB, hT2[:, sl],
                                 start=True, stop=True)
                nc.scalar.activation(e2[:, sl], s2[:], AF.Exp,
                                     accum_out=Sacc2[:, ch:ch + 1])
                ebc = prep.tile([128, 512], F32, tag="bc", name="ebc", bufs=2)
                nc.tensor.matmul(ebc[:], sel2, e2[:, sl],
                                 start=True, stop=True)
                # ztwT2 holds MINUS zt*e so the pooled numerator (T - ztw)
                # comes out of one subtract; Tacc accumulates -T.
                nc.vector.scalar_tensor_tensor(
                    out=ztwT2[:, sl], in0=ztT2[:, sl], scalar=-1.0,
                    in1=ebc[:], op0=ALU.mult, op1=ALU.mult,
                    accum_out=Tacc[:, ch:ch + 1])

            # ---------------- phase 2: pooled = (T - ztw)/(S - e) ----------
            nc.vector.reduce_sum(T2[:], Tacc[:], axis=AX.X)   # = -T
            nc.vector.reduce_sum(S2[:], Sacc2[:], axis=AX.X)
            nc.vector.tensor_copy(S2b[:], S2[:])
            Sp = prep.tile([128, 1], F32, tag="mi", name="Sp")
            nc.tensor.matmul(Sp[:], sel2, S2b[:], start=True, stop=True)
            nc.vector.tensor_copy(Sb[:], Sp[:])
            for ch in range(4):
                sl = slice(ch * 512, (ch + 1) * 512)
                bbc = prep.tile([128, 512], F32, tag="bc", name="bbc", bufs=2)
                nc.tensor.matmul(bbc[:], sel2, e2[:, sl],
                                 start=True, stop=True)
                # bcs1 = Relu(S - e) == S - e  (strictly positive)
                nc.scalar.activation(bcs1[:, sl], bbc[:], AF.Relu,
                                     scale=-1.0, bias=Sb[:])
                with nc.allow_low_precision(reason="beta bf16"):
                    nc.vector.reciprocal(bcs[:, sl], bcs1[:, sl])
                # numer = -ztw - (-T) = T - ztw;  pooled = numer/(S - e)
                nc.vector.tensor_scalar(numer[:, sl], ztwT2[:, sl],
                                        T2[:], None, op0=ALU.subtract)
                nc.vector.tensor_tensor(VA[0:64, sl], numer[0:64, sl],
                                        bcs[0:64, sl], op=ALU.mult)
                nc.vector.tensor_tensor(VB[64:128, sl], numer[64:128, sl],
                                        bcs[64:128, sl], op=ALU.mult)

        # ---------------- diagonal pieces (no PSUM needed) -----------------
        pid = nc.vector.partition_id()
        vsl = bass.ts(pid % 4, OWN)
        nc.vector.tensor_tensor(prodA[:], U0[:], VA[:, vsl], op=ALU.mult)
        nc.vector.tensor_tensor(prodB[:], U1[:], VB[:, vsl], op=ALU.mult)
        nc.gpsimd.reduce_sum(dout[0:1, 0:OWN], prodA[:], axis=AX.C)
        nc.gpsimd.reduce_sum(dout[0:1, OWN:2 * OWN], prodB[:], axis=AX.C)

        # ---------------- main loop: 8 tiles of [128 rows x 2048 cols] -----
        with tc.tile_pool(name="mainp", bufs=2, space="PSUM") as mainp:
            for t in range(8):
                ic, g = t // 2, t % 2
                U = U0 if g == 0 else U1
                V = VA if g == 0 else VB
                pm = mainp.tile([128, G], F32, tag="pb", name="pm")
                for q in range(4):
                    nc.tensor.matmul(pm[:, q * 512:(q + 1) * 512],
                                     U[:, ic * 128:(ic + 1) * 128],
                                     V[:, q * 512:(q + 1) * 512],
                                     start=True, stop=True)
                if t in SCHRAUD_TILES:
                    esi = scr.tile([128, G], I32, tag="esi", name="esi")
                    nc.vector.tensor_scalar(esi[:], pm[:],
                                            biasD[:, t:t + 1], 0.0,
                                            op0=ALU.add, op1=ALU.max)
                    nc.vector.reduce_sum(seacc[:, t:t + 1],
                                         esi[:].bitcast(F32), axis=AX.X)
                else:
                    es = scr.tile([128, G], BF16, tag="es", name="es")
                    nc.scalar.activation(es[:], pm[:], AF.Exp,
                                         bias=biasS[:, t:t + 1], scale=INV_A,
                                         accum_out=seacc[:, t:t + 1])

        nc.sync.dma_start(out1_d[:], out1[:])
        nc.sync.dma_start(dout_d[:], dout[:])

    nc.compile()
    return nc


_built = None


def _get_program():
    global _built
    if _built is None:
        _built = _build_program()
    return _built


def make_in_maps(inputs):
    import ml_dtypes
    BF = ml_dtypes.bfloat16
    f = lambda x: np.ascontiguousarray(np.asarray(x, dtype=np.float32))
    bf = lambda x: np.ascontiguousarray(np.asarray(x, np.float32).astype(BF))

    zw = np.concatenate([f(inputs['zw_0']), f(inputs['zw_1'])], axis=0)
    zwT = np.ascontiguousarray(zw.T)                      # [128, 4096]
    # chunk-packed: block ch = [g0 cols ch*512:+512 | g1 cols ch*512:+512]
    zwTbCH = np.empty((128, B), np.float32)
    for ch in range(4):
        zwTbCH[:, ch * 1024:ch * 1024 + 512] = zwT[:, ch * 512:(ch + 1) * 512]
        zwTbCH[:, ch * 1024 + 512:ch * 1024 + 1024] = \
            zwT[:, G + ch * 512:G + (ch + 1) * 512]

    wk = f(inputs['Wk_w'])
    b0 = f(inputs['Ww0_b']) + f(inputs['Wk_b'])
    b1 = f(inputs['Ww1_b']) + f(inputs['Wk_b'])

    a1wB = np.zeros((128, 64), np.float32)
    a1wB[0:64, 0:32] = f(inputs['a0_1w']).T
    a1wB[64:128, 32:64] = f(inputs['a1_1w']).T
    a2wB = np.zeros((64, 2), np.float32)
    a2wB[0:32, 0:1] = f(inputs['a0_2w']).T
    a2wB[32:64, 1:2] = f(inputs['a1_2w']).T
    sel2 = np.zeros((2, 128), np.float32)
    sel2[0, 0:64] = 1.0
    sel2[1, 64:128] = 1.0

    wbf = np.zeros((128, WBF), np.float32)
    wbf[:, 0:64] = f(inputs['lin0_w']).T
    wbf[:, 64:128] = f(inputs['lin1_w']).T
    wbf[:, 128:192] = a1wB
    wbf[0:64, 192:194] = a2wB
    wbf[0:2, 194:322] = sel2
    wbf[:, 322:323] = (A_SCALE * b0).reshape(128, 1)
    wbf[:, 323:324] = (A_SCALE * b1).reshape(128, 1)
    wbf[:, 324:452] = A_SCALE * np.hstack([f(inputs['Ww0_w']), wk])
    wbf[:, 452:580] = A_SCALE * np.hstack([wk, f(inputs['Ww1_w'])])

    wf32 = np.zeros((128, 2), np.float32)
    wf32[:, 0] = np.concatenate([f(inputs['lin0_b']), f(inputs['lin1_b'])])
    wf32[0:64, 1] = np.concatenate([f(inputs['a0_1b']), f(inputs['a1_1b'])])

    cT = f(inputs['c']).T                                  # [64, 4096]

    base = {
        'zwTb': bf(zwTbCH),
        'cT0': bf(cT[:, 0:G]),
        'cT1': bf(cT[:, G:B]),
        'wbf': bf(wbf),
        'wf32': wf32,
    }
    in_maps = []
    for cid in range(N_CORES):
        m = dict(base)
        m['zwoTb'] = bf(zwT[:, cid * OWN:(cid + 1) * OWN])
        in_maps.append(m)
    return in_maps


def kernel(**inputs):
    nc = _get_program()
    in_maps = make_in_maps(inputs)
    res = run_bass_kernel_spmd(nc, in_maps, list(range(N_CORES)))
    total = 0.0
    for cid, r in enumerate(res.results):
        g = cid // 4
        out1 = np.asarray(r['out1'], dtype=np.float64)   # [128,16]
        dout = np.asarray(r['dout'], dtype=np.float64).reshape(2, 512)
        seacc, biasS = out1[:, 0:8], out1[:, 8:16]
        diag_raw = dout[g, :] / A_SCALE                  # [512] own rows
        for ic in range(4):
            p = np.arange(128)
            u = ic * 128 + p
            sumexp = seacc[p, 2 * ic] + seacc[p, 2 * ic + 1]
            v = diag_raw[u] + biasS[p, 2 * ic + g] - np.log(sumexp)
            total += v.sum()
    return np.array(-(total / B), dtype=np.float32)
